# revision 1
# baseline (speedup 1.0000x reference)
"""CARAFE (content-aware upsample, power-normalized softmax) on 8 TRN2 cores.

Math (reference.py): X (2,256,64,64) ->
  conv1x1(256->64) + bn + relu -> conv3x3(64->100) + bn -> pixel_shuffle(2)
  -> W (2,25,128,128) -> softmax(clip(W)^p) over 25 taps
  out[b,c,y,x] = sum_{ki,kj} W[b,(ki,kj),y,x] * Xpad[b,c,y//2+ki-2,x//2+kj-2]

Strategy (pure data-parallel over h, 8 rows / core):
  * conv1x1 / conv3x3 as bf16 GEMMs (channels on partitions).
  * softmax via ACT transcendentals; tap-sum via a 100x4 selection matmul;
    reciprocal on a reshaped [128,16] tile.
  * The per-pixel 25-tap weighted sum is a banded matmul per output row h:
    out[c,(ry,x)] = sum_{p} XT_r[p,c] * B_ki[p,(ry,x)] accumulated over ki,
    where B_ki[p, col] = Wnorm[(ki,kj),...] iff p = x//2 + kj.  B is produced
    by writing the dense softmax output to a zero-padded DRAM scratch
    ("epad", donated-zero output buffer) and reading it back with a
    fused-stride DMA access pattern that materializes the banded layout
    (including the zeros) directly in SBUF.
  * XT_r strips come from PE transposes of the input rows.

kernel(**inputs) takes the FULL inputs and returns the FULL output.
"""

import numpy as np
import ml_dtypes

SCALE = 2
K_UP = 5
B, C, H, W = 2, 256, 64, 64
N_CORES = 8
HS = H // N_CORES            # 8 low-res rows per core
XROWS = HS + 4               # 12 rows (with +-2 halo)
WP = W + 4                   # 68 (w padded by 2 each side)
CMID, CENC = 64, 100
# epad: per (b,h,ki) a [64 w-blocks x 131 slots x 2 ry x 2 rx] zero-padded
# buffer; valid kj slots are 63..67 (slot = kj + ZOFF, kj = p - w).
NSLOT = 131
ZOFF = 63
SWB = 4 * NSLOT              # 524 elements per w-block
EPN = W * SWB                # 33536 elements per (b,h,ki)

_STATE = {}


def _build_nc():
    import concourse.bass as bass
    import concourse.tile as tile
    from concourse import mybir
    from concourse.vector_clock import ScopedClock
    from concourse.tile_rust import add_dep_helper

    # --- workaround: this walrus build rejects >1 sync-wait on CTRL-class
    # instructions; split the Tile tail-drain waits into 1-wait NOPs. ---
    def patched_drain_and_barrier(self, tick_clock, wait_clock):
        maxw = 1
        carrier = self.nc.sync.nop()
        wait_clock.add_sem_waits(
            carrier.ins, ScopedClock({None: tick_clock.global_clock})
        )
        si = carrier.ins.sync_info
        waits = list(si.on_wait) if si is not None else []
        if len(waits) > maxw:
            si.on_wait = waits[:maxw]
            carrier.ins.sync_info = si
            rest = waits[maxw:]
            for i in range(0, len(rest), maxw):
                n = self.nc.sync.nop()
                n.ins.sync_info = mybir.SyncInfo(
                    on_wait=rest[i : i + maxw], on_update=[]
                )
        self.nc.sync.drain()
        self.nc.all_engine_barrier()
        assert self.sems is not None
        popped = self.nc._tile_sem_poison_stack.pop()
        assert popped is self._sem_poison
        self.nc.clear_and_free_semaphores(list(self.sems.allocated().values()))
        self.nc.all_engine_barrier()

    tile.TileContext._drain_and_barrier = patched_drain_and_barrier

    # --- workaround #2: the same walrus build accepts at most ONE sync wait
    # on ANY instruction.  Post-process the serialized BIR: hoist excess
    # waits onto single-wait NoOps inserted just before, on the same engine
    # (same program point, so semantics are unchanged). ---
    import orjson

    def _split_waits_json(raw: bytes) -> bytes:
        j = orjson.loads(raw)
        n = 0
        changed = False
        for fn in j["functions"]:
            for bb in fn["blocks"]:
                out = []
                for ins in bb["instructions"]:
                    si = ins.get("sync_info")
                    waits = si.get("on_wait") if si else None
                    if waits and len(waits) > 1:
                        changed = True
                        for wt in waits[:-1]:
                            n += 1
                            out.append(
                                {
                                    "debug": ins.get("debug", 0),
                                    "engine": ins["engine"],
                                    "ins": [],
                                    "outs": [],
                                    "name": f"WSPL-{n}",
                                    "opcode": "NoOp",
                                    "sync_info": {"on_update": [], "on_wait": [wt]},
                                }
                            )
                        si["on_wait"] = [waits[-1]]
                    out.append(ins)
                bb["instructions"] = out
        return orjson.dumps(j) if changed else raw

    if not getattr(bass.Bass.to_json_bytes, "_wait_split", False):
        _orig_tjb = bass.Bass.to_json_bytes

        def patched_to_json_bytes(self):
            return _split_waits_json(_orig_tjb(self))

        patched_to_json_bytes._wait_split = True
        bass.Bass.to_json_bytes = patched_to_json_bytes

    f32 = mybir.dt.float32
    bf16 = mybir.dt.bfloat16
    AF = mybir.ActivationFunctionType

    nc = bass.Bass()

    # ---- parameters ----
    xh = nc.declare_dram_parameter("xh", [B, C, XROWS, WP], bf16, isOutput=False)
    comp_w = nc.declare_dram_parameter("comp_w", [CMID, C, 1, 1], f32, isOutput=False)
    c_g = nc.declare_dram_parameter("comp_gamma", [CMID], f32, isOutput=False)
    c_b = nc.declare_dram_parameter("comp_beta", [CMID], f32, isOutput=False)
    c_m = nc.declare_dram_parameter("comp_mean", [CMID], f32, isOutput=False)
    c_v = nc.declare_dram_parameter("comp_var", [CMID], f32, isOutput=False)
    enc_w = nc.declare_dram_parameter("enc_w", [CENC, CMID, 3, 3], f32, isOutput=False)
    e_g = nc.declare_dram_parameter("enc_gamma", [CENC], f32, isOutput=False)
    e_b = nc.declare_dram_parameter("enc_beta", [CENC], f32, isOutput=False)
    e_m = nc.declare_dram_parameter("enc_mean", [CENC], f32, isOutput=False)
    e_v = nc.declare_dram_parameter("enc_var", [CENC], f32, isOutput=False)
    p_in = nc.declare_dram_parameter("power_p", [1], f32, isOutput=False)
    sel = nc.declare_dram_parameter("sel", [CENC, 4], f32, isOutput=False)
    ident = nc.declare_dram_parameter("ident", [128, 128], bf16, isOutput=False)
    y1mask = nc.declare_dram_parameter("y1mask", [660], bf16, isOutput=False)

    out = nc.declare_dram_parameter(
        "out", [B, C, 2 * HS, 2 * W], f32, isOutput=True
    )
    # donated-zero scratch outputs (never read host-side)
    epad = nc.declare_dram_parameter("epad", [B, HS, K_UP, EPN], bf16, isOutput=True)
    rscr = nc.declare_dram_parameter("rscr", [B, 2048], f32, isOutput=True)
    pscr = nc.declare_dram_parameter("pscr", [1], f32, isOutput=True)

    def dram_ap(param, offset, dims):
        return bass.AP(tensor=param, offset=offset, ap=[list(d) for d in dims])

    with tile.TileContext(nc) as tc:
        import contextlib

        ctx = contextlib.ExitStack()
        const = ctx.enter_context(tc.tile_pool(name="const", bufs=1))
        stage = ctx.enter_context(tc.tile_pool(name="stage", bufs=2))
        sm = ctx.enter_context(tc.tile_pool(name="sm", bufs=2))
        xtp = ctx.enter_context(tc.tile_pool(name="xtp", bufs=24))
        bp = ctx.enter_context(tc.tile_pool(name="bp", bufs=12))
        op = ctx.enter_context(tc.tile_pool(name="op", bufs=4))
        ps_c1 = ctx.enter_context(tc.tile_pool(name="ps_c1", bufs=2, space="PSUM"))
        ps_c3 = ctx.enter_context(tc.tile_pool(name="ps_c3", bufs=1, space="PSUM"))
        ps_s = ctx.enter_context(tc.tile_pool(name="ps_s", bufs=1, space="PSUM"))
        ps_t = ctx.enter_context(tc.tile_pool(name="ps_t", bufs=2, space="PSUM"))
        ps_e = ctx.enter_context(tc.tile_pool(name="ps_e", bufs=2, space="PSUM"))

        # ---- constants in SBUF ----
        ident_sb = const.tile([128, 128], bf16, tag="ident")
        nc.sync.dma_start(out=ident_sb[:, :], in_=ident[:, :])
        sel_sb = const.tile([CENC, 4], f32, tag="sel")
        nc.sync.dma_start(out=sel_sb[:, :], in_=sel[:, :])
        mask_sb = const.tile([CMID, 10, 66], bf16, tag="mask")
        nc.sync.dma_start(
            out=mask_sb[:, :, :],
            in_=dram_ap(y1mask, 0, [[0, CMID], [66, 10], [1, 66]]),
        )

        # conv1x1 weights: lhsT [cin(128) x cout(64)] per cin-half
        comp_bf = []
        for ct in range(2):
            cf = stage.tile([128, CMID], f32, tag="wstage")
            nc.sync.dma_start(
                out=cf[:, :],
                in_=dram_ap(comp_w, ct * 128, [[1, 128], [C, CMID]]),
            )
            cb = const.tile([128, CMID], bf16, tag=f"comp_bf{ct}")
            nc.vector.tensor_copy(cb[:, :], cf[:, :])
            comp_bf.append(cb)

        # conv3x3 weights: lhsT [cin(64) x cout(100)] per (dy,dx)
        enc_bf = []
        for j in range(9):
            ef = stage.tile([CMID, CENC], f32, tag="wstage")
            nc.sync.dma_start(
                out=ef[:, :],
                in_=dram_ap(enc_w, j, [[9, CMID], [9 * CMID, CENC]]),
            )
            eb = const.tile([CMID, CENC], bf16, tag=f"enc_bf{j}")
            nc.vector.tensor_copy(eb[:, :], ef[:, :])
            enc_bf.append(eb)

        # ---- batchnorm fold: inv = gamma/sqrt(var+eps), shift = beta-mean*inv
        def bn_fold(gamma, beta, mean, var, n, tagp):
            g = const.tile([n, 1], f32, tag=f"{tagp}g")
            bt = const.tile([n, 1], f32, tag=f"{tagp}b")
            m = const.tile([n, 1], f32, tag=f"{tagp}m")
            v = const.tile([n, 1], f32, tag=f"{tagp}v")
            for t, src in ((g, gamma), (bt, beta), (m, mean), (v, var)):
                nc.sync.dma_start(out=t[:, :], in_=dram_ap(src, 0, [[1, n]]))
            eps = const.tile([n, 1], f32, tag=f"{tagp}e")
            nc.vector.memset(eps[:, :], 1e-5)
            std = const.tile([n, 1], f32, tag=f"{tagp}s")
            nc.scalar.activation(std[:, :], v[:, :], AF.Sqrt, bias=eps[:, :])
            rstd = const.tile([n, 1], f32, tag=f"{tagp}r")
            nc.vector.reciprocal(rstd[:, :], std[:, :])
            inv = const.tile([n, 1], f32, tag=f"{tagp}i")
            nc.vector.tensor_mul(inv[:, :], g[:, :], rstd[:, :])
            tmp = const.tile([n, 1], f32, tag=f"{tagp}t")
            nc.vector.tensor_mul(tmp[:, :], m[:, :], inv[:, :])
            shift = const.tile([n, 1], f32, tag=f"{tagp}h")
            nc.vector.tensor_sub(shift[:, :], bt[:, :], tmp[:, :])
            return inv, shift

        inv1, shift1 = bn_fold(c_g, c_b, c_m, c_v, CMID, "bn1")
        inv2, shift2 = bn_fold(e_g, e_b, e_m, e_v, CENC, "bn2")

        # ---- p = clip(power_p, 1e-5), broadcast to [100,1] via DRAM bounce
        p_sb = const.tile([1, 1], f32, tag="p")
        nc.sync.dma_start(out=p_sb[:, :], in_=dram_ap(p_in, 0, [[1, 1]]))
        nc.vector.tensor_scalar_max(p_sb[:, :], p_sb[:, :], 1e-5)
        p_wr = nc.sync.dma_start(out=dram_ap(pscr, 0, [[1, 1]]), in_=p_sb[:, :])
        pb_sb = const.tile([CENC, 1], f32, tag="pb")
        p_rd = nc.sync.dma_start(
            out=pb_sb[:, :], in_=dram_ap(pscr, 0, [[0, CENC], [1, 1]])
        )
        add_dep_helper(p_rd.ins, p_wr.ins, sync=True, reason="pscr RAW")

        # ---- X rows in SBUF (bf16, already padded on host) ----
        xbf = [[None, None] for _ in range(B)]
        for b in range(B):
            for ct in range(2):
                t = const.tile([128, XROWS, WP], bf16, tag=f"xbf{b}{ct}")
                nc.sync.dma_start(
                    out=t[:, :, :],
                    in_=dram_ap(
                        xh,
                        b * C * XROWS * WP + ct * 128 * XROWS * WP,
                        [[XROWS * WP, 128], [1, XROWS * WP]],
                    ),
                )
                xbf[b][ct] = t

        # ---- Y1 tiles (zeroed once; borders stay zero) ----
        y1 = []
        for b in range(B):
            t = const.tile([CMID, 10, 66], bf16, tag=f"y1_{b}")
            nc.vector.memset(t[:, :, :], 0.0)
            y1.append(t)

        epad_wr = {}  # (b,h) -> dma inst
        xts = {}  # (b,j) -> X^T strip tile

        for b in range(B):
            # ===== conv1x1 + bn1 + relu =====
            for half in range(2):
                pc = ps_c1.tile([CMID, 320], f32, tag="c1")
                for ct in range(2):
                    nc.tensor.matmul(
                        pc[:, :],
                        comp_bf[ct][:, :],
                        xbf[b][ct][:, 1 + 5 * half : 6 + 5 * half, 2 : 2 + W],
                        start=(ct == 0),
                        stop=(ct == 1),
                    )
                nc.scalar.activation(
                    y1[b][:, 5 * half : 5 * half + 5, 1 : 1 + W],
                    pc[:, :],
                    AF.Relu,
                    bias=shift1[:, :],
                    scale=inv1[:, :],
                )
            # zero out-of-image rows / padding cols
            nc.vector.tensor_mul(y1[b][:, :, :], y1[b][:, :, :], mask_sb[:, :, :])

            # ===== conv3x3 + bn2 =====
            pc3 = ps_c3.tile([CENC, HS * W], f32, tag="c3")
            jj = 0
            for dy in (-1, 0, 1):
                for dx in (-1, 0, 1):
                    nc.tensor.matmul(
                        pc3[:, :],
                        enc_bf[jj][:, :],
                        y1[b][:, 1 + dy : 9 + dy, 1 + dx : 1 + dx + W],
                        start=(jj == 0),
                        stop=(jj == 8),
                    )
                    jj += 1
            w_sb = sm.tile([CENC, HS * W], f32, tag="w")
            nc.scalar.activation(
                w_sb[:, :], pc3[:, :], AF.Identity, bias=shift2[:, :], scale=inv2[:, :]
            )

            # ===== power + softmax numerator =====
            nc.vector.tensor_scalar_max(w_sb[:, :], w_sb[:, :], 1e-5)
            nc.scalar.activation(w_sb[:, :], w_sb[:, :], AF.Ln)
            nc.scalar.activation(w_sb[:, :], w_sb[:, :], AF.Exp, scale=pb_sb[:, :])
            e_sb = sm.tile([CENC, HS * W], f32, tag="e")
            nc.scalar.activation(e_sb[:, :], w_sb[:, :], AF.Exp)

            # ===== tap-sums, reciprocal, normalize =====
            ps = ps_s.tile([4, HS * W], f32, tag="s")
            nc.tensor.matmul(ps[:, :], sel_sb[:, :], e_sb[:, :], start=True, stop=True)
            sums_sb = sm.tile([4, 32, 16], f32, tag="sums")
            nc.vector.tensor_copy(sums_sb[:, :, :], ps[:, :].rearrange("p (a b) -> p a b", b=16))
            s128 = sm.tile([128, 16], f32, tag="s128")
            nc.sync.dma_start(out=s128[:, :], in_=sums_sb[:, :, :])
            r128 = sm.tile([128, 16], f32, tag="r128")
            nc.vector.reciprocal(r128[:, :], s128[:, :])
            r_wr = nc.sync.dma_start(
                out=dram_ap(rscr, b * 2048, [[16, 128], [1, 16]]), in_=r128[:, :]
            )
            rb_sb = sm.tile([CENC, HS * W], f32, tag="rb")
            r_rd = nc.sync.dma_start(
                out=rb_sb[:, :],
                in_=dram_ap(rscr, b * 2048, [[0, 25], [512, 4], [1, 512]]),
            )
            add_dep_helper(r_rd.ins, r_wr.ins, sync=True, reason="rscr RAW")
            en_sb = sm.tile([CENC, HS, W], bf16, tag="en")
            nc.vector.tensor_mul(
                en_sb[:, :, :],
                e_sb[:, :].rearrange("p (a b) -> p a b", b=W),
                rb_sb[:, :].rearrange("p (a b) -> p a b", b=W),
            )

            # ===== scatter normalized weights to banded DRAM scratch =====
            # per ki: src [20 partitions, 64]; dst slots (kj,ry,rx) are the
            # 20 contiguous elements at ZOFF*4, strided SWB per w-block.
            for h in range(HS):
                base = (b * HS + h) * K_UP * EPN
                wrs = []
                for ki in range(K_UP):
                    wr = nc.sync.dma_start(
                        out=dram_ap(
                            epad,
                            base + ki * EPN + ZOFF * 4,
                            [[1, 20], [SWB, W]],
                        ),
                        in_=en_sb[20 * ki : 20 * (ki + 1), h, :],
                    )
                    wrs.append(wr)
                epad_wr[(b, h)] = wrs

            # ===== X^T strips via PE transpose =====
            for j in range(XROWS):
                pt = ps_t.tile([WP, 256], bf16, tag="pt")
                for ct in range(2):
                    nc.tensor.transpose(
                        pt[:, ct * 128 : (ct + 1) * 128],
                        xbf[b][ct][:, j, :],
                        ident_sb[:, :],
                    )
                xt = xtp.tile([WP, 256], bf16, tag="xt")
                nc.any.tensor_copy(xt[:, :], pt[:, :])
                xts[(b, j)] = xt

            # ===== banded einsum =====
            for h in range(HS):
                bts = []
                for ki in range(K_UP):
                    bt = bp.tile([WP, 256], bf16, tag="bt")
                    for ry in range(2):
                        rd = nc.sync.dma_start(
                            out=bt[:, ry * 128 : (ry + 1) * 128],
                            in_=dram_ap(
                                epad,
                                (b * HS + h) * K_UP * EPN
                                + ki * EPN
                                + ZOFF * 4
                                + 2 * ry,
                                [[4, WP], [SWB - 4, W], [1, 2]],
                            ),
                        )
                        add_dep_helper(
                            rd.ins,
                            epad_wr[(b, h)][ki].ins,
                            sync=True,
                            reason="epad RAW",
                        )
                    bts.append(bt)
                for ct in range(2):
                    pe = ps_e.tile([128, 256], f32, tag="pe")
                    for ki in range(K_UP):
                        nc.tensor.matmul(
                            pe[:, :],
                            xts[(b, h + ki)][:, ct * 128 : (ct + 1) * 128],
                            bts[ki][:, :],
                            start=(ki == 0),
                            stop=(ki == K_UP - 1),
                        )
                    o_sb = op.tile([128, 256], f32, tag="osb")
                    nc.any.tensor_copy(o_sb[:, :], pe[:, :])
                    nc.sync.dma_start(
                        out=dram_ap(
                            out,
                            b * C * 2 * HS * 2 * W
                            + ct * 128 * 2 * HS * 2 * W
                            + 2 * h * 2 * W,
                            [[2 * HS * 2 * W, 128], [2 * W, 2], [1, 2 * W]],
                        ),
                        in_=o_sb[:, :],
                    )

        ctx.close()

    return nc


def _get_nc():
    if "nc" not in _STATE:
        _STATE["nc"] = _build_nc()
    return _STATE["nc"]


def _make_in_maps(inputs):
    X = np.asarray(inputs["X"], dtype=np.float32)
    Xp = np.pad(X, ((0, 0), (0, 0), (2, 2), (2, 2)))
    sel = np.zeros((CENC, 4), np.float32)
    for p in range(CENC):
        sel[p, p % 4] = 1.0
    ident = np.eye(128, dtype=ml_dtypes.bfloat16)
    common = {
        "comp_w": np.asarray(inputs["comp_w"], np.float32),
        "comp_gamma": np.asarray(inputs["comp_gamma"], np.float32),
        "comp_beta": np.asarray(inputs["comp_beta"], np.float32),
        "comp_mean": np.asarray(inputs["comp_mean"], np.float32),
        "comp_var": np.asarray(inputs["comp_var"], np.float32),
        "enc_w": np.asarray(inputs["enc_w"], np.float32),
        "enc_gamma": np.asarray(inputs["enc_gamma"], np.float32),
        "enc_beta": np.asarray(inputs["enc_beta"], np.float32),
        "enc_mean": np.asarray(inputs["enc_mean"], np.float32),
        "enc_var": np.asarray(inputs["enc_var"], np.float32),
        "power_p": np.asarray(inputs["power_p"], np.float32),
        "sel": sel,
        "ident": ident,
    }
    in_maps = []
    for core in range(N_CORES):
        r0 = HS * core
        xh = np.ascontiguousarray(Xp[:, :, r0 : r0 + XROWS, :]).astype(
            ml_dtypes.bfloat16
        )
        mask = np.zeros((10, 66), np.float32)
        for rr in range(10):
            grow = r0 - 1 + rr
            if 0 <= grow < H:
                mask[rr, 1 : 1 + W] = 1.0
        m = dict(common)
        m["xh"] = xh
        m["y1mask"] = mask.reshape(660).astype(ml_dtypes.bfloat16)
        in_maps.append(m)
    return in_maps


def _run(inputs, trace=False):
    from concourse.bass_utils import run_bass_kernel_spmd

    if trace:
        import sys, os
        sys.path.insert(0, os.path.dirname(os.path.abspath(__file__)))
        import hookshim  # noqa: F401

    nc = _get_nc()
    in_maps = _make_in_maps(inputs)
    res = run_bass_kernel_spmd(
        nc, in_maps, core_ids=list(range(N_CORES)), trace=trace
    )
    out = np.concatenate([res.results[c]["out"] for c in range(N_CORES)], axis=2)
    return out, res


def kernel(**inputs):
    out, _ = _run(inputs, trace=False)
    return out



# revision 8
# speedup vs baseline: 11.3578x; 11.3578x over previous
"""CARAFE (content-aware upsample, power-normalized softmax) on 8 TRN2 cores.

Math (reference.py): X (2,256,64,64) ->
  conv1x1(256->64) + bn + relu -> conv3x3(64->100) + bn -> pixel_shuffle(2)
  -> W (2,25,128,128) -> softmax(clip(W)^p) over 25 taps
  out[b,c,y,x] = sum_{ki,kj} W[b,(ki,kj),y,x] * Xpad[b,c,y//2+ki-2,x//2+kj-2]

Strategy (pure data-parallel over h, 8 rows / core):
  * conv1x1 / conv3x3 as bf16 GEMMs (channels on partitions); weights are
    host-pretransposed to lhsT layout in bf16 so every load is contiguous.
  * enc output channels are host-reordered/padded to c' = 32*s + k (s =
    subpixel (ry,rx), k = tap) so the softmax works on aligned 32-partition
    groups.
  * softmax via ACT transcendentals; tap-sum via a 128x4 selection matmul;
    reciprocal directly on the [4, 512] sums.
  * The per-pixel 25-tap weighted sum is a banded matmul per output row h:
    out[c,(x,s)] = sum_p XT[p,c] * B_ki[p,(x,s)] accumulated over ki, where
    B_ki[p, (x,s)] = Wnorm[(ki,kj=p-x),s] at (h,x), zero outside the band.
    B for ALL 8 h-rows of a batch is materialized via one zero-padded DRAM
    scratch block per (b,ki) with the flat layout
      addr(kj,x,s,h) = 2048*kj + 2080*x + 8*s + h
    which is collision-free, gives 64-byte scatter-write runs ((s,h)
    innermost, 10 write DMAs total) and a fully contiguous readback
    (bt[p, 0:2048] = 4KB runs, 2 read DMAs total).
  * X^T strips are DMAed from a host-pretransposed copy (xt input) - no PE
    transposes.
  * Outputs accumulate into one [128, 2048] tile per (b,ct) -> 4 big out
    DMAs.  DMA issue is split between SP and ACT (both HWDGE) to halve the
    per-instruction issue serialization.

kernel(**inputs) takes the FULL inputs and returns the FULL output.
"""

import numpy as np
import ml_dtypes

SCALE = 2
K_UP = 5
B, C, H, W = 2, 256, 64, 64
N_CORES = 8
HS = H // N_CORES            # 8 low-res rows per core
XROWS = HS + 4               # 12 rows (with +-2 halo)
WP = W + 4                   # 68 (w padded by 2 each side)
CMID, CENC = 64, 100
CENCP = 128                  # enc channels padded: c' = 32*s + k (k<25)
# epad: per (b,ki) a flat EPK-element zero-padded block; element (kj,x,s,h)
# lives at 2048*kj + 2080*x + 8*s + h (collision-free).  Readback for
# partition p is epad[2048*p : 2048*p + 2048] -> contiguous.
EPK = 2048 * WP              # 139264 elements per (b,ki)

_STATE = {}


def _build_nc():
    import concourse.bass as bass
    import concourse.tile as tile
    from concourse import mybir
    from concourse.vector_clock import ScopedClock
    from concourse.tile_rust import add_dep_helper

    # --- workaround: this walrus build rejects >1 sync-wait on CTRL-class
    # instructions; split the Tile tail-drain waits into 1-wait NOPs. ---
    def patched_drain_and_barrier(self, tick_clock, wait_clock):
        maxw = 1
        carrier = self.nc.sync.nop()
        wait_clock.add_sem_waits(
            carrier.ins, ScopedClock({None: tick_clock.global_clock})
        )
        si = carrier.ins.sync_info
        waits = list(si.on_wait) if si is not None else []
        if len(waits) > maxw:
            si.on_wait = waits[:maxw]
            carrier.ins.sync_info = si
            rest = waits[maxw:]
            for i in range(0, len(rest), maxw):
                n = self.nc.sync.nop()
                n.ins.sync_info = mybir.SyncInfo(
                    on_wait=rest[i : i + maxw], on_update=[]
                )
        self.nc.sync.drain()
        self.nc.all_engine_barrier()
        assert self.sems is not None
        popped = self.nc._tile_sem_poison_stack.pop()
        assert popped is self._sem_poison
        self.nc.clear_and_free_semaphores(list(self.sems.allocated().values()))
        self.nc.all_engine_barrier()

    tile.TileContext._drain_and_barrier = patched_drain_and_barrier

    # --- workaround #2: the same walrus build accepts at most ONE sync wait
    # on ANY instruction.  Post-process the serialized BIR: hoist excess
    # waits onto single-wait NoOps inserted just before, on the same engine
    # (same program point, so semantics are unchanged). ---
    import orjson

    def _split_waits_json(raw: bytes) -> bytes:
        j = orjson.loads(raw)
        n = 0
        changed = False
        for fn in j["functions"]:
            for bb in fn["blocks"]:
                out = []
                for ins in bb["instructions"]:
                    si = ins.get("sync_info")
                    waits = si.get("on_wait") if si else None
                    if waits and len(waits) > 1:
                        changed = True
                        for wt in waits[:-1]:
                            n += 1
                            out.append(
                                {
                                    "debug": ins.get("debug", 0),
                                    "engine": ins["engine"],
                                    "ins": [],
                                    "outs": [],
                                    "name": f"WSPL-{n}",
                                    "opcode": "NoOp",
                                    "sync_info": {"on_update": [], "on_wait": [wt]},
                                }
                            )
                        si["on_wait"] = [waits[-1]]
                    out.append(ins)
                bb["instructions"] = out
        return orjson.dumps(j) if changed else raw

    if not getattr(bass.Bass.to_json_bytes, "_wait_split", False):
        _orig_tjb = bass.Bass.to_json_bytes

        def patched_to_json_bytes(self):
            return _split_waits_json(_orig_tjb(self))

        patched_to_json_bytes._wait_split = True
        bass.Bass.to_json_bytes = patched_to_json_bytes

    f32 = mybir.dt.float32
    bf16 = mybir.dt.bfloat16
    AF = mybir.ActivationFunctionType

    nc = bass.Bass()

    # ---- parameters ----
    xh = nc.declare_dram_parameter("xh", [B, 2, 128, XROWS * WP], bf16, isOutput=False)
    xt = nc.declare_dram_parameter("xt", [B, XROWS, WP, C], bf16, isOutput=False)
    comp_wt = nc.declare_dram_parameter("comp_wt", [2, 128, CMID], bf16, isOutput=False)
    c_g = nc.declare_dram_parameter("comp_gamma", [CMID], f32, isOutput=False)
    c_b = nc.declare_dram_parameter("comp_beta", [CMID], f32, isOutput=False)
    c_m = nc.declare_dram_parameter("comp_mean", [CMID], f32, isOutput=False)
    c_v = nc.declare_dram_parameter("comp_var", [CMID], f32, isOutput=False)
    enc_wt = nc.declare_dram_parameter("enc_wt", [9, CMID, CENCP], bf16, isOutput=False)
    e_g = nc.declare_dram_parameter("enc_gamma", [CENCP], f32, isOutput=False)
    e_b = nc.declare_dram_parameter("enc_beta", [CENCP], f32, isOutput=False)
    e_m = nc.declare_dram_parameter("enc_mean", [CENCP], f32, isOutput=False)
    e_v = nc.declare_dram_parameter("enc_var", [CENCP], f32, isOutput=False)
    p_in = nc.declare_dram_parameter("power_p", [1], f32, isOutput=False)
    sel = nc.declare_dram_parameter("sel", [CENCP, 4], f32, isOutput=False)
    y1mask = nc.declare_dram_parameter("y1mask", [660], bf16, isOutput=False)

    out = nc.declare_dram_parameter(
        "out", [B, C, 2 * HS, 2 * W], f32, isOutput=True
    )
    # donated-zero scratch outputs (never read host-side)
    epad = nc.declare_dram_parameter("epad", [B, K_UP, EPK], bf16, isOutput=True)
    rscr = nc.declare_dram_parameter("rscr", [B, 2048], f32, isOutput=True)
    pscr = nc.declare_dram_parameter("pscr", [1], f32, isOutput=True)

    def dram_ap(param, offset, dims):
        return bass.AP(tensor=param, offset=offset, ap=[list(d) for d in dims])

    with tile.TileContext(nc) as tc:
        import contextlib

        ctx = contextlib.ExitStack()
        const = ctx.enter_context(tc.tile_pool(name="const", bufs=1))
        sm = ctx.enter_context(tc.tile_pool(name="sm", bufs=2))
        bp = ctx.enter_context(tc.tile_pool(name="bp", bufs=2))
        op = ctx.enter_context(tc.tile_pool(name="op", bufs=2))
        ps_c1 = ctx.enter_context(tc.tile_pool(name="ps_c1", bufs=2, space="PSUM"))
        ps_c3 = ctx.enter_context(tc.tile_pool(name="ps_c3", bufs=1, space="PSUM"))
        ps_s = ctx.enter_context(tc.tile_pool(name="ps_s", bufs=1, space="PSUM"))
        ps_e = ctx.enter_context(tc.tile_pool(name="ps_e", bufs=2, space="PSUM"))

        # ---- constants in SBUF (ACT issues const loads) ----
        sel_sb = const.tile([CENCP, 4], f32, tag="sel")
        nc.scalar.dma_start(out=sel_sb[:, :], in_=sel[:, :])
        mask_sb = const.tile([CMID, 10, 66], bf16, tag="mask")
        nc.scalar.dma_start(
            out=mask_sb[:, :, :],
            in_=dram_ap(y1mask, 0, [[0, CMID], [66, 10], [1, 66]]),
        )
        comp_all = const.tile([128, 2, CMID], bf16, tag="comp")
        nc.scalar.dma_start(
            out=comp_all[:, :, :],
            in_=dram_ap(comp_wt, 0, [[CMID, 128], [128 * CMID, 2], [1, CMID]]),
        )
        enc_all = const.tile([CMID, 9, CENCP], bf16, tag="enc")
        nc.scalar.dma_start(
            out=enc_all[:, :, :],
            in_=dram_ap(enc_wt, 0, [[CENCP, CMID], [CMID * CENCP, 9], [1, CENCP]]),
        )

        # ---- batchnorm fold: inv = gamma/sqrt(var+eps), shift = beta-mean*inv
        def bn_fold(gamma, beta, mean, var, n, tagp):
            g = const.tile([n, 1], f32, tag=f"{tagp}g")
            bt = const.tile([n, 1], f32, tag=f"{tagp}b")
            m = const.tile([n, 1], f32, tag=f"{tagp}m")
            v = const.tile([n, 1], f32, tag=f"{tagp}v")
            for t, src in ((g, gamma), (bt, beta), (m, mean), (v, var)):
                nc.scalar.dma_start(out=t[:, :], in_=dram_ap(src, 0, [[1, n]]))
            eps = const.tile([n, 1], f32, tag=f"{tagp}e")
            nc.vector.memset(eps[:, :], 1e-5)
            std = const.tile([n, 1], f32, tag=f"{tagp}s")
            nc.scalar.activation(std[:, :], v[:, :], AF.Sqrt, bias=eps[:, :])
            rstd = const.tile([n, 1], f32, tag=f"{tagp}r")
            nc.vector.reciprocal(rstd[:, :], std[:, :])
            inv = const.tile([n, 1], f32, tag=f"{tagp}i")
            nc.vector.tensor_mul(inv[:, :], g[:, :], rstd[:, :])
            tmp = const.tile([n, 1], f32, tag=f"{tagp}t")
            nc.vector.tensor_mul(tmp[:, :], m[:, :], inv[:, :])
            shift = const.tile([n, 1], f32, tag=f"{tagp}h")
            nc.vector.tensor_sub(shift[:, :], bt[:, :], tmp[:, :])
            return inv, shift

        inv1, shift1 = bn_fold(c_g, c_b, c_m, c_v, CMID, "bn1")
        inv2, shift2 = bn_fold(e_g, e_b, e_m, e_v, CENCP, "bn2")

        # ---- p = clip(power_p, 1e-5), broadcast to [128,1] via DRAM bounce
        p_sb = const.tile([1, 1], f32, tag="p")
        nc.scalar.dma_start(out=p_sb[:, :], in_=dram_ap(p_in, 0, [[1, 1]]))
        nc.vector.tensor_scalar_max(p_sb[:, :], p_sb[:, :], 1e-5)
        p_wr = nc.scalar.dma_start(out=dram_ap(pscr, 0, [[1, 1]]), in_=p_sb[:, :])
        pb_sb = const.tile([CENCP, 1], f32, tag="pb")
        p_rd = nc.scalar.dma_start(
            out=pb_sb[:, :], in_=dram_ap(pscr, 0, [[0, CENCP], [1, 1]])
        )
        add_dep_helper(p_rd.ins, p_wr.ins, sync=True, reason="pscr RAW")

        # ---- X rows in SBUF (bf16, already padded on host) ----
        xbf = []
        for b in range(B):
            t = const.tile([128, 2, XROWS, WP], bf16, tag=f"xbf{b}")
            nc.scalar.dma_start(
                out=t[:, :, :, :],
                in_=dram_ap(
                    xh,
                    b * 2 * 128 * XROWS * WP,
                    [[XROWS * WP, 128], [128 * XROWS * WP, 2], [1, XROWS * WP]],
                ),
            )
            xbf.append(t)

        # ---- X^T strips straight from host-transposed xt ----
        xts = []
        for b in range(B):
            t = const.tile([WP, XROWS, C], bf16, tag=f"xts{b}")
            nc.scalar.dma_start(
                out=t[:, :, :],
                in_=dram_ap(
                    xt,
                    b * XROWS * WP * C,
                    [[C, WP], [WP * C, XROWS], [1, C]],
                ),
            )
            xts.append(t)

        # ---- Y1 tiles (zeroed once; borders stay zero) ----
        y1 = []
        for b in range(B):
            t = const.tile([CMID, 10, 66], bf16, tag=f"y1_{b}")
            nc.vector.memset(t[:, :, :], 0.0)
            y1.append(t)

        epad_wr = {}  # b -> [dma inst per ki]

        # ===== PHASE 1 (per batch): conv -> softmax -> scatter =====
        for b in range(B):
            # conv1x1 + bn1 + relu
            for half in range(2):
                pc = ps_c1.tile([CMID, 320], f32, tag="c1")
                for ct in range(2):
                    nc.tensor.matmul(
                        pc[:, :],
                        comp_all[:, ct, :],
                        xbf[b][:, ct, 1 + 5 * half : 6 + 5 * half, 2 : 2 + W],
                        start=(ct == 0),
                        stop=(ct == 1),
                    )
                nc.scalar.activation(
                    y1[b][:, 5 * half : 5 * half + 5, 1 : 1 + W],
                    pc[:, :],
                    AF.Relu,
                    bias=shift1[:, :],
                    scale=inv1[:, :],
                )
            # zero out-of-image rows / padding cols
            nc.vector.tensor_mul(y1[b][:, :, :], y1[b][:, :, :], mask_sb[:, :, :])

            # conv3x3 + bn2
            pc3 = ps_c3.tile([CENCP, HS * W], f32, tag="c3")
            jj = 0
            for dy in (-1, 0, 1):
                for dx in (-1, 0, 1):
                    nc.tensor.matmul(
                        pc3[:, :],
                        enc_all[:, jj, :],
                        y1[b][:, 1 + dy : 9 + dy, 1 + dx : 1 + dx + W],
                        start=(jj == 0),
                        stop=(jj == 8),
                    )
                    jj += 1
            w_sb = sm.tile([CENCP, HS * W], f32, tag="w")
            nc.scalar.activation(
                w_sb[:, :], pc3[:, :], AF.Identity, bias=shift2[:, :], scale=inv2[:, :]
            )

            # power + softmax numerator
            nc.vector.tensor_scalar_max(w_sb[:, :], w_sb[:, :], 1e-5)
            nc.scalar.activation(w_sb[:, :], w_sb[:, :], AF.Ln)
            nc.scalar.activation(w_sb[:, :], w_sb[:, :], AF.Exp, scale=pb_sb[:, :])
            e_sb = sm.tile([CENCP, HS, W], f32, tag="e")
            nc.scalar.activation(e_sb[:, :, :], w_sb[:, :], AF.Exp)

            # tap-sums (per subpixel s), reciprocal, broadcast back
            ps = ps_s.tile([4, HS * W], f32, tag="s")
            nc.tensor.matmul(
                ps[:, :], sel_sb[:, :], e_sb[:, :, :], start=True, stop=True
            )
            r4 = sm.tile([4, HS * W], f32, tag="r4")
            nc.vector.reciprocal(r4[:, :], ps[:, :])
            r_wr = nc.sync.dma_start(
                out=dram_ap(rscr, b * 2048, [[512, 4], [1, 512]]), in_=r4[:, :]
            )
            rb_sb = sm.tile([CENCP, HS, W], f32, tag="rb")
            r_rd = nc.sync.dma_start(
                out=rb_sb[:, :, :],
                in_=dram_ap(rscr, b * 2048, [[512, 4], [0, 32], [1, 512]]),
            )
            add_dep_helper(r_rd.ins, r_wr.ins, sync=True, reason="rscr RAW")

            # normalized weights folded to [25 taps, (x, s, h)] layout
            en25 = sm.tile([25, W, 4, HS], bf16, tag="en25")
            for s in range(4):
                nc.vector.tensor_mul(
                    en25[:, :, s, :],
                    e_sb[32 * s : 32 * s + 25, :, :].rearrange("p h x -> p x h"),
                    rb_sb[32 * s : 32 * s + 25, :, :].rearrange("p h x -> p x h"),
                )

            # scatter to banded DRAM scratch (64B runs: (s,h) innermost)
            wrs = []
            for ki in range(K_UP):
                wr = nc.sync.dma_start(
                    out=dram_ap(
                        epad,
                        (b * K_UP + ki) * EPK,
                        [[2048, K_UP], [2080, W], [1, 32]],
                    ),
                    in_=en25[5 * ki : 5 * (ki + 1), :, :, :],
                )
                wrs.append(wr)
            epad_wr[b] = wrs

        # ===== PHASE 2 (per batch): banded matmuls -> output =====
        for b in range(B):
            bt_all = bp.tile([WP, K_UP, W, 4, HS], bf16, tag="bt")
            rd = nc.sync.dma_start(
                out=bt_all[:, :, :, :, :],
                in_=dram_ap(
                    epad,
                    b * K_UP * EPK,
                    [[2048, WP], [EPK, K_UP], [1, 2048]],
                ),
            )
            for wr in epad_wr[b]:
                add_dep_helper(rd.ins, wr.ins, sync=True, reason="epad RAW")
            for ct in range(2):
                o_all = op.tile([128, HS, 2, W, 2], f32, tag="oall")
                for h in range(HS):
                    pe = ps_e.tile([128, 256], f32, tag="pe")
                    for ki in range(K_UP):
                        nc.tensor.matmul(
                            pe[:, :],
                            xts[b][:, h + ki, ct * 128 : (ct + 1) * 128],
                            bt_all[:, ki, :, :, h],
                            start=(ki == 0),
                            stop=(ki == K_UP - 1),
                        )
                    # psum cols are (x,ry,rx); store as (ry,x,rx)
                    nc.any.tensor_copy(
                        o_all[:, h, :, :, :],
                        pe[:, :].rearrange("p (x ry rx) -> p ry x rx", ry=2, rx=2),
                    )
                nc.scalar.dma_start(
                    out=dram_ap(
                        out,
                        (b * C + ct * 128) * 2 * HS * 2 * W,
                        [[2 * HS * 2 * W, 128], [1, 2 * HS * 2 * W]],
                    ),
                    in_=o_all[:, :, :, :, :],
                )

        ctx.close()

    return nc


def _get_nc():
    if "nc" not in _STATE:
        _STATE["nc"] = _build_nc()
    return _STATE["nc"]


def _make_in_maps(inputs):
    X = np.asarray(inputs["X"], dtype=np.float32)
    Xp = np.pad(X, ((0, 0), (0, 0), (2, 2), (2, 2)))

    # enc output channels reordered+padded: new row c' = 32*s + k holds old
    # channel 4*k + s for k < 25; rows with k >= 25 are zero padding.
    def _pad_enc(a, fill=0.0):
        shp = (CENCP,) + a.shape[1:]
        o = np.full(shp, fill, np.float32)
        for s in range(4):
            for k in range(25):
                o[32 * s + k] = a[4 * k + s]
        return o

    sel = np.zeros((CENCP, 4), np.float32)
    for s in range(4):
        sel[32 * s : 32 * s + 25, s] = 1.0

    # lhsT weight layouts, bf16
    cw = np.asarray(inputs["comp_w"], np.float32)[:, :, 0, 0]   # [64, 256]
    comp_wt = np.ascontiguousarray(cw.T.reshape(2, 128, CMID)).astype(
        ml_dtypes.bfloat16
    )
    ew = _pad_enc(np.asarray(inputs["enc_w"], np.float32))      # [128, 64, 3, 3]
    enc_wt = np.ascontiguousarray(
        ew.reshape(CENCP, CMID, 9).transpose(2, 1, 0)
    ).astype(ml_dtypes.bfloat16)                                # [9, 64, 128]

    common = {
        "comp_wt": comp_wt,
        "comp_gamma": np.asarray(inputs["comp_gamma"], np.float32),
        "comp_beta": np.asarray(inputs["comp_beta"], np.float32),
        "comp_mean": np.asarray(inputs["comp_mean"], np.float32),
        "comp_var": np.asarray(inputs["comp_var"], np.float32),
        "enc_wt": enc_wt,
        "enc_gamma": _pad_enc(np.asarray(inputs["enc_gamma"], np.float32), fill=1.0),
        "enc_beta": _pad_enc(np.asarray(inputs["enc_beta"], np.float32)),
        "enc_mean": _pad_enc(np.asarray(inputs["enc_mean"], np.float32)),
        "enc_var": _pad_enc(np.asarray(inputs["enc_var"], np.float32), fill=1.0),
        "power_p": np.asarray(inputs["power_p"], np.float32),
        "sel": sel,
    }
    in_maps = []
    for core in range(N_CORES):
        r0 = HS * core
        win = Xp[:, :, r0 : r0 + XROWS, :]
        xhv = np.ascontiguousarray(win).astype(ml_dtypes.bfloat16)
        xtv = np.ascontiguousarray(win.transpose(0, 2, 3, 1)).astype(
            ml_dtypes.bfloat16
        )
        mask = np.zeros((10, 66), np.float32)
        for rr in range(10):
            grow = r0 - 1 + rr
            if 0 <= grow < H:
                mask[rr, 1 : 1 + W] = 1.0
        m = dict(common)
        m["xh"] = xhv.reshape(B, 2, 128, XROWS * WP)
        m["xt"] = xtv
        m["y1mask"] = mask.reshape(660).astype(ml_dtypes.bfloat16)
        in_maps.append(m)
    return in_maps


def _run(inputs, trace=False):
    from concourse.bass_utils import run_bass_kernel_spmd

    if trace:
        import sys, os
        sys.path.insert(0, os.path.dirname(os.path.abspath(__file__)))
        import hookshim  # noqa: F401

    nc = _get_nc()
    in_maps = _make_in_maps(inputs)
    res = run_bass_kernel_spmd(
        nc, in_maps, core_ids=list(range(N_CORES)), trace=trace
    )
    out = np.concatenate([res.results[c]["out"] for c in range(N_CORES)], axis=2)
    return out, res


def kernel(**inputs):
    out, _ = _run(inputs, trace=False)
    return out


# revision 11
# speedup vs baseline: 12.4714x; 1.0981x over previous
"""CARAFE (content-aware upsample, power-normalized softmax) on 8 TRN2 cores.

Math (reference.py): X (2,256,64,64) ->
  conv1x1(256->64) + bn + relu -> conv3x3(64->100) + bn -> pixel_shuffle(2)
  -> W (2,25,128,128) -> softmax(clip(W)^p) over 25 taps
  out[b,c,y,x] = sum_{ki,kj} W[b,(ki,kj),y,x] * Xpad[b,c,y//2+ki-2,x//2+kj-2]

Strategy (pure data-parallel over h, 8 rows / core):
  * conv1x1 / conv3x3 as bf16 GEMMs (channels on partitions); weights are
    host-pretransposed to lhsT layout in bf16 (contiguous loads).
  * enc output channels host-reordered/padded to c' = 32*s + k (s = subpixel
    (ry,rx), k = tap) so softmax groups are aligned 32-partition slices.
  * softmax: ACT transcendentals in bf16; tap-sums via a [128x4] selection
    matmul; 1/S via the ACT Reciprocal LUT on [4,512]; the reciprocals are
    broadcast back to all 128 partitions with a tiny [4->128] PE matmul
    (no DRAM bounce anywhere in this chain).
  * The per-pixel 25-tap weighted sum is a banded matmul per output row h:
    out[c,(x,s)] = sum_p XT[p,c] * B_ki[p,(x,s)] accumulated over ki, where
    B_ki[p,(x,s)] = Wnorm[(ki,kj=p-x),s] at (h,x), zero outside the band.
    B for ALL 8 h-rows of a batch lives in a zero-padded DRAM scratch block
    per (b,ki) with flat layout addr(kj,x,s,h) = 2048*kj + 2080*x + 8*s + h
    (collision-free): scatter writes have 64B runs (5 DMAs per batch),
    readback is fully contiguous 4KB runs (1 DMA per (b,ki), so the first
    banded matmul only waits for the ki=0 block).
  * Banded matmuls run ki-innermost over h-pairs into a [128,2,256] PSUM
    tile; X^T strips come from a host-pretransposed copy (xt input).
  * Outputs accumulate into a [128, 2048] tile per (b,ct), written out in
    two 512KB DMAs.  DMA issue is split between SP (inputs, band traffic)
    and ACT (consts, outputs) - both HWDGE.

kernel(**inputs) takes the FULL inputs and returns the FULL output.
"""

import numpy as np
import ml_dtypes

SCALE = 2
K_UP = 5
B, C, H, W = 2, 256, 64, 64
N_CORES = 8
HS = H // N_CORES            # 8 low-res rows per core
XROWS = HS + 4               # 12 rows (with +-2 halo)
WP = W + 4                   # 68 (w padded by 2 each side)
CMID, CENC = 64, 100
CENCP = 128                  # enc channels padded: c' = 32*s + k (k<25)
# epad: per (b,ki) a flat EPK-element zero-padded block; element (kj,x,s,h)
# lives at 2048*kj + 2080*x + 8*s + h (collision-free).  Readback for
# partition p is epad[2048*p : 2048*p + 2048] -> contiguous.
EPK = 2048 * WP              # 139264 elements per (b,ki)

_STATE = {}


def _build_nc():
    import concourse.bass as bass
    import concourse.tile as tile
    from concourse import mybir
    from concourse.vector_clock import ScopedClock
    from concourse.tile_rust import add_dep_helper

    # --- workaround: this walrus build rejects >1 sync-wait on CTRL-class
    # instructions; split the Tile tail-drain waits into 1-wait NOPs. ---
    def patched_drain_and_barrier(self, tick_clock, wait_clock):
        maxw = 1
        carrier = self.nc.sync.nop()
        wait_clock.add_sem_waits(
            carrier.ins, ScopedClock({None: tick_clock.global_clock})
        )
        si = carrier.ins.sync_info
        waits = list(si.on_wait) if si is not None else []
        if len(waits) > maxw:
            si.on_wait = waits[:maxw]
            carrier.ins.sync_info = si
            rest = waits[maxw:]
            for i in range(0, len(rest), maxw):
                n = self.nc.sync.nop()
                n.ins.sync_info = mybir.SyncInfo(
                    on_wait=rest[i : i + maxw], on_update=[]
                )
        self.nc.sync.drain()
        self.nc.all_engine_barrier()
        assert self.sems is not None
        popped = self.nc._tile_sem_poison_stack.pop()
        assert popped is self._sem_poison
        self.nc.clear_and_free_semaphores(list(self.sems.allocated().values()))
        self.nc.all_engine_barrier()

    tile.TileContext._drain_and_barrier = patched_drain_and_barrier

    # --- workaround #2: the same walrus build accepts at most ONE sync wait
    # on ANY instruction.  Post-process the serialized BIR: hoist excess
    # waits onto single-wait NoOps inserted just before, on the same engine
    # (same program point, so semantics are unchanged). ---
    import orjson

    def _split_waits_json(raw: bytes) -> bytes:
        j = orjson.loads(raw)
        n = 0
        changed = False
        for fn in j["functions"]:
            for bb in fn["blocks"]:
                out = []
                for ins in bb["instructions"]:
                    si = ins.get("sync_info")
                    waits = si.get("on_wait") if si else None
                    if waits and len(waits) > 1:
                        changed = True
                        for wt in waits[:-1]:
                            n += 1
                            out.append(
                                {
                                    "debug": ins.get("debug", 0),
                                    "engine": ins["engine"],
                                    "ins": [],
                                    "outs": [],
                                    "name": f"WSPL-{n}",
                                    "opcode": "NoOp",
                                    "sync_info": {"on_update": [], "on_wait": [wt]},
                                }
                            )
                        si["on_wait"] = [waits[-1]]
                    out.append(ins)
                bb["instructions"] = out
        return orjson.dumps(j) if changed else raw

    if not getattr(bass.Bass.to_json_bytes, "_wait_split", False):
        _orig_tjb = bass.Bass.to_json_bytes

        def patched_to_json_bytes(self):
            return _split_waits_json(_orig_tjb(self))

        patched_to_json_bytes._wait_split = True
        bass.Bass.to_json_bytes = patched_to_json_bytes

    f32 = mybir.dt.float32
    bf16 = mybir.dt.bfloat16
    AF = mybir.ActivationFunctionType

    nc = bass.Bass()

    # ---- parameters ----
    xh = nc.declare_dram_parameter("xh", [B, 2, 128, XROWS * WP], bf16, isOutput=False)
    xt = nc.declare_dram_parameter("xt", [B, XROWS, WP, C], bf16, isOutput=False)
    comp_wt = nc.declare_dram_parameter("comp_wt", [2, 128, CMID], bf16, isOutput=False)
    c_g = nc.declare_dram_parameter("comp_gamma", [CMID], f32, isOutput=False)
    c_b = nc.declare_dram_parameter("comp_beta", [CMID], f32, isOutput=False)
    c_m = nc.declare_dram_parameter("comp_mean", [CMID], f32, isOutput=False)
    c_v = nc.declare_dram_parameter("comp_var", [CMID], f32, isOutput=False)
    enc_wt = nc.declare_dram_parameter("enc_wt", [9, CMID, CENCP], bf16, isOutput=False)
    e_g = nc.declare_dram_parameter("enc_gamma", [CENCP], f32, isOutput=False)
    e_b = nc.declare_dram_parameter("enc_beta", [CENCP], f32, isOutput=False)
    e_m = nc.declare_dram_parameter("enc_mean", [CENCP], f32, isOutput=False)
    e_v = nc.declare_dram_parameter("enc_var", [CENCP], f32, isOutput=False)
    p_in = nc.declare_dram_parameter("power_p", [1], f32, isOutput=False)
    sel = nc.declare_dram_parameter("sel", [CENCP, 4], bf16, isOutput=False)
    bc4 = nc.declare_dram_parameter("bc4", [4, 128], bf16, isOutput=False)
    y1mask = nc.declare_dram_parameter("y1mask", [660], bf16, isOutput=False)

    out = nc.declare_dram_parameter(
        "out", [B, C, 2 * HS, 2 * W], f32, isOutput=True
    )
    # donated-zero scratch outputs (never read host-side)
    epad = nc.declare_dram_parameter("epad", [B, K_UP, EPK], bf16, isOutput=True)
    pscr = nc.declare_dram_parameter("pscr", [1], f32, isOutput=True)

    def dram_ap(param, offset, dims):
        return bass.AP(tensor=param, offset=offset, ap=[list(d) for d in dims])

    with tile.TileContext(nc) as tc:
        import contextlib

        ctx = contextlib.ExitStack()
        const = ctx.enter_context(tc.tile_pool(name="const", bufs=1))
        sm = ctx.enter_context(tc.tile_pool(name="sm", bufs=2))
        bp = ctx.enter_context(tc.tile_pool(name="bp", bufs=6))
        op = ctx.enter_context(tc.tile_pool(name="op", bufs=2))
        ps_c1 = ctx.enter_context(tc.tile_pool(name="ps_c1", bufs=2, space="PSUM"))
        ps_c3 = ctx.enter_context(tc.tile_pool(name="ps_c3", bufs=1, space="PSUM"))
        ps_s = ctx.enter_context(tc.tile_pool(name="ps_s", bufs=1, space="PSUM"))
        ps_rb = ctx.enter_context(tc.tile_pool(name="ps_rb", bufs=1, space="PSUM"))
        ps_e = ctx.enter_context(tc.tile_pool(name="ps_e", bufs=3, space="PSUM"))

        # ---- inputs first (SP queue) so conv can start ASAP ----
        xbf, xts = [], []
        for b in range(B):
            t = const.tile([128, 2, XROWS, WP], bf16, tag=f"xbf{b}")
            nc.sync.dma_start(
                out=t[:, :, :, :],
                in_=dram_ap(
                    xh,
                    b * 2 * 128 * XROWS * WP,
                    [[XROWS * WP, 128], [128 * XROWS * WP, 2], [1, XROWS * WP]],
                ),
            )
            xbf.append(t)
            t2 = const.tile([WP, XROWS, C], bf16, tag=f"xts{b}")
            nc.sync.dma_start(
                out=t2[:, :, :],
                in_=dram_ap(
                    xt,
                    b * XROWS * WP * C,
                    [[C, WP], [WP * C, XROWS], [1, C]],
                ),
            )
            xts.append(t2)

        # ---- constants in SBUF (ACT queue) ----
        sel_sb = const.tile([CENCP, 4], bf16, tag="sel")
        nc.scalar.dma_start(out=sel_sb[:, :], in_=sel[:, :])
        bc4_sb = const.tile([4, 128], bf16, tag="bc4")
        nc.scalar.dma_start(out=bc4_sb[:, :], in_=bc4[:, :])
        mask_sb = const.tile([CMID, 10, 66], bf16, tag="mask")
        nc.scalar.dma_start(
            out=mask_sb[:, :, :],
            in_=dram_ap(y1mask, 0, [[0, CMID], [66, 10], [1, 66]]),
        )
        comp_all = const.tile([128, 2, CMID], bf16, tag="comp")
        nc.scalar.dma_start(
            out=comp_all[:, :, :],
            in_=dram_ap(comp_wt, 0, [[CMID, 128], [128 * CMID, 2], [1, CMID]]),
        )
        enc_all = const.tile([CMID, 9, CENCP], bf16, tag="enc")
        nc.scalar.dma_start(
            out=enc_all[:, :, :],
            in_=dram_ap(enc_wt, 0, [[CENCP, CMID], [CMID * CENCP, 9], [1, CENCP]]),
        )

        # ---- batchnorm fold: inv = gamma/sqrt(var+eps), shift = beta-mean*inv
        def bn_fold(gamma, beta, mean, var, n, tagp):
            g = const.tile([n, 1], f32, tag=f"{tagp}g")
            bt = const.tile([n, 1], f32, tag=f"{tagp}b")
            m = const.tile([n, 1], f32, tag=f"{tagp}m")
            v = const.tile([n, 1], f32, tag=f"{tagp}v")
            for t, src in ((g, gamma), (bt, beta), (m, mean), (v, var)):
                nc.scalar.dma_start(out=t[:, :], in_=dram_ap(src, 0, [[1, n]]))
            eps = const.tile([n, 1], f32, tag=f"{tagp}e")
            nc.vector.memset(eps[:, :], 1e-5)
            std = const.tile([n, 1], f32, tag=f"{tagp}s")
            nc.scalar.activation(std[:, :], v[:, :], AF.Sqrt, bias=eps[:, :])
            rstd = const.tile([n, 1], f32, tag=f"{tagp}r")
            nc.vector.reciprocal(rstd[:, :], std[:, :])
            inv = const.tile([n, 1], f32, tag=f"{tagp}i")
            nc.vector.tensor_mul(inv[:, :], g[:, :], rstd[:, :])
            tmp = const.tile([n, 1], f32, tag=f"{tagp}t")
            nc.vector.tensor_mul(tmp[:, :], m[:, :], inv[:, :])
            shift = const.tile([n, 1], f32, tag=f"{tagp}h")
            nc.vector.tensor_sub(shift[:, :], bt[:, :], tmp[:, :])
            return inv, shift

        inv1, shift1 = bn_fold(c_g, c_b, c_m, c_v, CMID, "bn1")
        inv2, shift2 = bn_fold(e_g, e_b, e_m, e_v, CENCP, "bn2")

        # ---- p = clip(power_p, 1e-5), broadcast to [128,1] via DRAM bounce
        p_sb = const.tile([1, 1], f32, tag="p")
        nc.scalar.dma_start(out=p_sb[:, :], in_=dram_ap(p_in, 0, [[1, 1]]))
        nc.vector.tensor_scalar_max(p_sb[:, :], p_sb[:, :], 1e-5)
        p_wr = nc.scalar.dma_start(out=dram_ap(pscr, 0, [[1, 1]]), in_=p_sb[:, :])
        pb_sb = const.tile([CENCP, 1], f32, tag="pb")
        p_rd = nc.scalar.dma_start(
            out=pb_sb[:, :], in_=dram_ap(pscr, 0, [[0, CENCP], [1, 1]])
        )
        add_dep_helper(p_rd.ins, p_wr.ins, sync=True, reason="pscr RAW")

        # ---- Y1 tiles (zeroed once; borders stay zero) ----
        y1 = []
        for b in range(B):
            t = const.tile([CMID, 10, 66], bf16, tag=f"y1_{b}")
            nc.vector.memset(t[:, :, :], 0.0)
            y1.append(t)

        epad_wr = {}  # (b,ki) -> dma inst

        # ===== PHASE 1 (per batch): conv -> softmax -> scatter =====
        for b in range(B):
            # conv1x1 + bn1 + relu
            for half in range(2):
                pc = ps_c1.tile([CMID, 320], f32, tag="c1")
                for ct in range(2):
                    nc.tensor.matmul(
                        pc[:, :],
                        comp_all[:, ct, :],
                        xbf[b][:, ct, 1 + 5 * half : 6 + 5 * half, 2 : 2 + W],
                        start=(ct == 0),
                        stop=(ct == 1),
                    )
                nc.scalar.activation(
                    y1[b][:, 5 * half : 5 * half + 5, 1 : 1 + W],
                    pc[:, :],
                    AF.Relu,
                    bias=shift1[:, :],
                    scale=inv1[:, :],
                )
            # zero out-of-image rows / padding cols
            nc.vector.tensor_mul(y1[b][:, :, :], y1[b][:, :, :], mask_sb[:, :, :])

            # conv3x3 + bn2
            pc3 = ps_c3.tile([CENCP, HS * W], f32, tag="c3")
            jj = 0
            for dy in (-1, 0, 1):
                for dx in (-1, 0, 1):
                    nc.tensor.matmul(
                        pc3[:, :],
                        enc_all[:, jj, :],
                        y1[b][:, 1 + dy : 9 + dy, 1 + dx : 1 + dx + W],
                        start=(jj == 0),
                        stop=(jj == 8),
                    )
                    jj += 1
            w_sb = sm.tile([CENCP, HS * W], f32, tag="w")
            nc.scalar.activation(
                w_sb[:, :], pc3[:, :], AF.Identity, bias=shift2[:, :], scale=inv2[:, :]
            )

            # power + softmax numerator (bf16 tail)
            nc.vector.tensor_scalar_max(w_sb[:, :], w_sb[:, :], 1e-5)
            nc.scalar.activation(w_sb[:, :], w_sb[:, :], AF.Ln)
            nc.scalar.activation(w_sb[:, :], w_sb[:, :], AF.Exp, scale=pb_sb[:, :])
            e_sb = sm.tile([CENCP, HS, W], bf16, tag="e")
            nc.scalar.activation(e_sb[:, :, :], w_sb[:, :], AF.Exp)

            # tap-sums (per subpixel s) -> 1/S via ACT LUT -> PE broadcast to
            # all 128 partitions (rb_ps[32s+k, :] = 1/S[s, :])
            ps = ps_s.tile([4, HS * W], f32, tag="s")
            nc.tensor.matmul(
                ps[:, :], sel_sb[:, :], e_sb[:, :, :], start=True, stop=True
            )
            l4 = sm.tile([4, HS * W], f32, tag="l4")
            nc.scalar.activation(l4[:, :], ps[:, :], AF.Ln)
            r4 = sm.tile([4, HS * W], bf16, tag="r4")
            nc.scalar.activation(r4[:, :], l4[:, :], AF.Exp, scale=-1.0)
            rb_ps = ps_rb.tile([CENCP, HS, W], f32, tag="rb")
            nc.tensor.matmul(
                rb_ps[:, :, :], bc4_sb[:, :], r4[:, :], start=True, stop=True
            )

            # normalized weights folded to [25 taps, (x, s, h)] layout
            en25 = sm.tile([25, W, 4, HS], bf16, tag="en25")
            for s in range(4):
                nc.vector.tensor_mul(
                    en25[:, :, s, :],
                    e_sb[32 * s : 32 * s + 25, :, :].rearrange("p h x -> p x h"),
                    rb_ps[32 * s : 32 * s + 25, :, :].rearrange("p h x -> p x h"),
                )

            # scatter to banded DRAM scratch (64B runs: (s,h) innermost)
            for ki in range(K_UP):
                wr = nc.sync.dma_start(
                    out=dram_ap(
                        epad,
                        (b * K_UP + ki) * EPK,
                        [[2048, K_UP], [2080, W], [1, 32]],
                    ),
                    in_=en25[5 * ki : 5 * (ki + 1), :, :, :],
                )
                epad_wr[(b, ki)] = wr

        # ===== PHASE 2 (per batch): banded matmuls -> output =====
        for b in range(B):
            bts = []
            for ki in range(K_UP):
                bt = bp.tile([WP, W, 4, HS], bf16, tag="bt")
                rd = nc.sync.dma_start(
                    out=bt[:, :, :, :],
                    in_=dram_ap(
                        epad,
                        (b * K_UP + ki) * EPK,
                        [[2048, WP], [1, 2048]],
                    ),
                )
                add_dep_helper(
                    rd.ins, epad_wr[(b, ki)].ins, sync=True, reason="epad RAW"
                )
                bts.append(bt)
            for ct in range(2):
                o_all = op.tile([128, HS, 2, W, 2], f32, tag="oall")
                for h in range(HS):
                    pe = ps_e.tile([128, 256], f32, tag="pe")
                    for ki in range(K_UP):
                        nc.tensor.matmul(
                            pe[:, :],
                            xts[b][:, h + ki, ct * 128 : (ct + 1) * 128],
                            bts[ki][:, :, :, h],
                            start=(ki == 0),
                            stop=(ki == K_UP - 1),
                        )
                    # psum cols are (x,ry,rx); store as (ry,x,rx)
                    nc.any.tensor_copy(
                        o_all[:, h, :, :, :],
                        pe[:, :].rearrange("p (x ry rx) -> p ry x rx", ry=2, rx=2),
                    )
                    if h == 3 or h == 7:
                        half = h // 4
                        nc.scalar.dma_start(
                            out=dram_ap(
                                out,
                                (b * C + ct * 128) * 2 * HS * 2 * W + half * 1024,
                                [[2 * HS * 2 * W, 128], [1, 1024]],
                            ),
                            in_=o_all[:, 4 * half : 4 * half + 4, :, :, :],
                        )

        ctx.close()

    return nc


def _get_nc():
    if "nc" not in _STATE:
        _STATE["nc"] = _build_nc()
    return _STATE["nc"]


def _make_in_maps(inputs):
    X = np.asarray(inputs["X"], dtype=np.float32)
    Xp = np.pad(X, ((0, 0), (0, 0), (2, 2), (2, 2)))

    # enc output channels reordered+padded: new row c' = 32*s + k holds old
    # channel 4*k + s for k < 25; rows with k >= 25 are zero padding.
    def _pad_enc(a, fill=0.0):
        shp = (CENCP,) + a.shape[1:]
        o = np.full(shp, fill, np.float32)
        for s in range(4):
            for k in range(25):
                o[32 * s + k] = a[4 * k + s]
        return o

    sel = np.zeros((CENCP, 4), np.float32)
    for s in range(4):
        sel[32 * s : 32 * s + 25, s] = 1.0
    bc4 = np.zeros((4, 128), np.float32)
    for s in range(4):
        bc4[s, 32 * s : 32 * (s + 1)] = 1.0

    # lhsT weight layouts, bf16
    cw = np.asarray(inputs["comp_w"], np.float32)[:, :, 0, 0]   # [64, 256]
    comp_wt = np.ascontiguousarray(cw.T.reshape(2, 128, CMID)).astype(
        ml_dtypes.bfloat16
    )
    ew = _pad_enc(np.asarray(inputs["enc_w"], np.float32))      # [128, 64, 3, 3]
    enc_wt = np.ascontiguousarray(
        ew.reshape(CENCP, CMID, 9).transpose(2, 1, 0)
    ).astype(ml_dtypes.bfloat16)                                # [9, 64, 128]

    common = {
        "comp_wt": comp_wt,
        "comp_gamma": np.asarray(inputs["comp_gamma"], np.float32),
        "comp_beta": np.asarray(inputs["comp_beta"], np.float32),
        "comp_mean": np.asarray(inputs["comp_mean"], np.float32),
        "comp_var": np.asarray(inputs["comp_var"], np.float32),
        "enc_wt": enc_wt,
        "enc_gamma": _pad_enc(np.asarray(inputs["enc_gamma"], np.float32), fill=1.0),
        "enc_beta": _pad_enc(np.asarray(inputs["enc_beta"], np.float32)),
        "enc_mean": _pad_enc(np.asarray(inputs["enc_mean"], np.float32)),
        "enc_var": _pad_enc(np.asarray(inputs["enc_var"], np.float32), fill=1.0),
        "power_p": np.asarray(inputs["power_p"], np.float32),
        "sel": sel.astype(ml_dtypes.bfloat16),
        "bc4": bc4.astype(ml_dtypes.bfloat16),
    }
    in_maps = []
    for core in range(N_CORES):
        r0 = HS * core
        win = Xp[:, :, r0 : r0 + XROWS, :]
        xhv = np.ascontiguousarray(win).astype(ml_dtypes.bfloat16)
        xtv = np.ascontiguousarray(win.transpose(0, 2, 3, 1)).astype(
            ml_dtypes.bfloat16
        )
        mask = np.zeros((10, 66), np.float32)
        for rr in range(10):
            grow = r0 - 1 + rr
            if 0 <= grow < H:
                mask[rr, 1 : 1 + W] = 1.0
        m = dict(common)
        m["xh"] = xhv.reshape(B, 2, 128, XROWS * WP)
        m["xt"] = xtv
        m["y1mask"] = mask.reshape(660).astype(ml_dtypes.bfloat16)
        in_maps.append(m)
    return in_maps


def _run(inputs, trace=False):
    from concourse.bass_utils import run_bass_kernel_spmd

    if trace:
        import sys, os
        sys.path.insert(0, os.path.dirname(os.path.abspath(__file__)))
        import hookshim  # noqa: F401

    nc = _get_nc()
    in_maps = _make_in_maps(inputs)
    res = run_bass_kernel_spmd(
        nc, in_maps, core_ids=list(range(N_CORES)), trace=trace
    )
    out = np.concatenate([res.results[c]["out"] for c in range(N_CORES)], axis=2)
    return out, res


def kernel(**inputs):
    out, _ = _run(inputs, trace=False)
    return out


# revision 17
# speedup vs baseline: 15.7435x; 1.2624x over previous
"""CARAFE (content-aware upsample, power-normalized softmax) on 8 TRN2 cores.

Math (reference.py): X (2,256,64,64) ->
  conv1x1(256->64) + bn + relu -> conv3x3(64->100) + bn -> pixel_shuffle(2)
  -> W (2,25,128,128) -> softmax(clip(W)^p) over 25 taps
  out[b,c,y,x] = sum_{ki,kj} W[b,(ki,kj),y,x] * Xpad[b,c,y//2+ki-2,x//2+kj-2]

Strategy (pure data-parallel over h, 8 rows / core):
  * conv1x1 / conv3x3 as bf16 GEMMs (channels on partitions); weights are
    host-pretransposed to lhsT layout in bf16 (contiguous loads).
  * enc output channels host-reordered/padded to c' = 32*s + k (s = subpixel
    (ry,rx), k = tap) so softmax groups are aligned 32-partition slices.
  * softmax: ACT transcendentals in bf16; tap-sums via a [128x4] selection
    matmul; 1/S via the ACT Reciprocal LUT on [4,512]; the reciprocals are
    broadcast back to all 128 partitions with a tiny [4->128] PE matmul
    (no DRAM bounce anywhere in this chain).
  * The per-pixel 25-tap weighted sum is a banded matmul per output row h:
    out[c,(x,s)] = sum_p XT[p,c] * B_ki[p,(x,s)] accumulated over ki, where
    B_ki[p,(x,s)] = Wnorm[(ki,kj=p-x),s] at (h,x), zero outside the band.
    B for ALL 8 h-rows of a batch lives in a zero-padded DRAM scratch block
    per (b,ki) with flat layout addr(kj,x,s,h) = 2048*kj + 2080*x + 8*s + h
    (collision-free): scatter writes have 64B runs (5 DMAs per batch),
    readback is fully contiguous 4KB runs (1 DMA per (b,ki), so the first
    banded matmul only waits for the ki=0 block).
  * Banded matmuls run ki-innermost over h-pairs into a [128,2,256] PSUM
    tile; X^T strips come from a host-pretransposed copy (xt input).
  * Outputs accumulate into a [128, 2048] tile per (b,ct), written out in
    two 512KB DMAs.  DMA issue is split between SP (inputs, band traffic)
    and ACT (consts, outputs) - both HWDGE.

kernel(**inputs) takes the FULL inputs and returns the FULL output.
"""

import numpy as np
import ml_dtypes

SCALE = 2
K_UP = 5
B, C, H, W = 2, 256, 64, 64
N_CORES = 8
HS = H // N_CORES            # 8 low-res rows per core
XROWS = HS + 4               # 12 rows (with +-2 halo)
WP = W + 4                   # 68 (w padded by 2 each side)
CMID, CENC = 64, 100
CENCP = 128                  # enc channels padded: c' = 32*s + k (k<25)
# epad: per (b,ki) a flat EPK-element zero-padded block; element (kj,x,s,h)
# lives at 2048*kj + 2080*x + 8*s + h (collision-free).  Readback for
# partition p is epad[2048*p : 2048*p + 2048] -> contiguous.
EPK = 2048 * WP              # 139264 elements per (b,ki)

_STATE = {}


def _build_nc():
    import concourse.bass as bass
    import concourse.tile as tile
    from concourse import mybir
    from concourse.vector_clock import ScopedClock
    from concourse.tile_rust import add_dep_helper

    # --- workaround: this walrus build rejects >1 sync-wait on CTRL-class
    # instructions; split the Tile tail-drain waits into 1-wait NOPs. ---
    def patched_drain_and_barrier(self, tick_clock, wait_clock):
        maxw = 1
        carrier = self.nc.sync.nop()
        wait_clock.add_sem_waits(
            carrier.ins, ScopedClock({None: tick_clock.global_clock})
        )
        si = carrier.ins.sync_info
        waits = list(si.on_wait) if si is not None else []
        if len(waits) > maxw:
            si.on_wait = waits[:maxw]
            carrier.ins.sync_info = si
            rest = waits[maxw:]
            for i in range(0, len(rest), maxw):
                n = self.nc.sync.nop()
                n.ins.sync_info = mybir.SyncInfo(
                    on_wait=rest[i : i + maxw], on_update=[]
                )
        self.nc.sync.drain()
        self.nc.all_engine_barrier()
        assert self.sems is not None
        popped = self.nc._tile_sem_poison_stack.pop()
        assert popped is self._sem_poison
        self.nc.clear_and_free_semaphores(list(self.sems.allocated().values()))
        self.nc.all_engine_barrier()

    tile.TileContext._drain_and_barrier = patched_drain_and_barrier

    # --- workaround #2: the same walrus build accepts at most ONE sync wait
    # on ANY instruction.  Post-process the serialized BIR: hoist excess
    # waits onto single-wait NoOps inserted just before, on the same engine
    # (same program point, so semantics are unchanged). ---
    import orjson

    def _split_waits_json(raw: bytes) -> bytes:
        j = orjson.loads(raw)
        n = 0
        changed = False
        for fn in j["functions"]:
            for bb in fn["blocks"]:
                out = []
                for ins in bb["instructions"]:
                    si = ins.get("sync_info")
                    waits = si.get("on_wait") if si else None
                    if waits and len(waits) > 1:
                        changed = True
                        for wt in waits[:-1]:
                            n += 1
                            out.append(
                                {
                                    "debug": ins.get("debug", 0),
                                    "engine": ins["engine"],
                                    "ins": [],
                                    "outs": [],
                                    "name": f"WSPL-{n}",
                                    "opcode": "NoOp",
                                    "sync_info": {"on_update": [], "on_wait": [wt]},
                                }
                            )
                        si["on_wait"] = [waits[-1]]
                    out.append(ins)
                bb["instructions"] = out
        return orjson.dumps(j) if changed else raw

    if not getattr(bass.Bass.to_json_bytes, "_wait_split", False):
        _orig_tjb = bass.Bass.to_json_bytes

        def patched_to_json_bytes(self):
            return _split_waits_json(_orig_tjb(self))

        patched_to_json_bytes._wait_split = True
        bass.Bass.to_json_bytes = patched_to_json_bytes

    f32 = mybir.dt.float32
    bf16 = mybir.dt.bfloat16
    AF = mybir.ActivationFunctionType

    nc = bass.Bass()

    # ---- parameters ----
    xh = nc.declare_dram_parameter("xh", [B, 2, 128, XROWS * WP], bf16, isOutput=False)
    xt = nc.declare_dram_parameter("xt", [B, XROWS, WP, C], bf16, isOutput=False)
    comp_wt = nc.declare_dram_parameter("comp_wt", [2, 128, CMID], bf16, isOutput=False)
    c_g = nc.declare_dram_parameter("comp_gamma", [CMID], f32, isOutput=False)
    c_b = nc.declare_dram_parameter("comp_beta", [CMID], f32, isOutput=False)
    c_m = nc.declare_dram_parameter("comp_mean", [CMID], f32, isOutput=False)
    c_v = nc.declare_dram_parameter("comp_var", [CMID], f32, isOutput=False)
    enc_wt = nc.declare_dram_parameter("enc_wt", [9, CMID, CENCP], bf16, isOutput=False)
    e_g = nc.declare_dram_parameter("enc_gamma", [CENCP], f32, isOutput=False)
    e_b = nc.declare_dram_parameter("enc_beta", [CENCP], f32, isOutput=False)
    e_m = nc.declare_dram_parameter("enc_mean", [CENCP], f32, isOutput=False)
    e_v = nc.declare_dram_parameter("enc_var", [CENCP], f32, isOutput=False)
    p_in = nc.declare_dram_parameter("power_p", [1], f32, isOutput=False)
    sel = nc.declare_dram_parameter("sel", [CENCP, 4], bf16, isOutput=False)
    bc4 = nc.declare_dram_parameter("bc4", [4, 128], bf16, isOutput=False)
    y1mask = nc.declare_dram_parameter("y1mask", [660], bf16, isOutput=False)

    out = nc.declare_dram_parameter(
        "out", [B, C, 2 * HS, 2 * W], f32, isOutput=True
    )
    # donated-zero scratch outputs (never read host-side)
    epad = nc.declare_dram_parameter("epad", [B, K_UP, EPK], bf16, isOutput=True)
    pscr = nc.declare_dram_parameter("pscr", [1], f32, isOutput=True)

    def dram_ap(param, offset, dims):
        return bass.AP(tensor=param, offset=offset, ap=[list(d) for d in dims])

    with tile.TileContext(nc) as tc:
        import contextlib

        ctx = contextlib.ExitStack()
        const = ctx.enter_context(tc.tile_pool(name="const", bufs=1))
        sm = ctx.enter_context(tc.tile_pool(name="sm", bufs=2))
        bp = ctx.enter_context(tc.tile_pool(name="bp", bufs=3))
        bc = ctx.enter_context(tc.tile_pool(name="bc", bufs=1))
        op = ctx.enter_context(tc.tile_pool(name="op", bufs=2))
        ps_c1 = ctx.enter_context(tc.tile_pool(name="ps_c1", bufs=2, space="PSUM"))
        ps_c3 = ctx.enter_context(tc.tile_pool(name="ps_c3", bufs=1, space="PSUM"))
        ps_s = ctx.enter_context(tc.tile_pool(name="ps_s", bufs=1, space="PSUM"))
        ps_rb = ctx.enter_context(tc.tile_pool(name="ps_rb", bufs=1, space="PSUM"))
        ps_e = ctx.enter_context(tc.tile_pool(name="ps_e", bufs=3, space="PSUM"))

        # ---- inputs first (SP queue) so conv can start ASAP; X^T strips
        # are only needed by phase 2, so they load after both xh images ----
        xbf, xts = [], []
        for b in range(B):
            t = const.tile([128, 2, XROWS, WP], bf16, tag=f"xbf{b}")
            nc.sync.dma_start(
                out=t[:, :, :, :],
                in_=dram_ap(
                    xh,
                    b * 2 * 128 * XROWS * WP,
                    [[XROWS * WP, 128], [128 * XROWS * WP, 2], [1, XROWS * WP]],
                ),
            )
            xbf.append(t)
        for b in range(B):
            t2 = const.tile([WP, XROWS, C], bf16, tag=f"xts{b}")
            nc.sync.dma_start(
                out=t2[:, :, :],
                in_=dram_ap(
                    xt,
                    b * XROWS * WP * C,
                    [[C, WP], [WP * C, XROWS], [1, C]],
                ),
            )
            xts.append(t2)

        # ---- constants in SBUF (ACT queue) ----
        sel_sb = const.tile([CENCP, 4], bf16, tag="sel")
        nc.scalar.dma_start(out=sel_sb[:, :], in_=sel[:, :])
        bc4_sb = const.tile([4, 128], bf16, tag="bc4")
        nc.scalar.dma_start(out=bc4_sb[:, :], in_=bc4[:, :])
        mask_sb = const.tile([CMID, 10, 66], bf16, tag="mask")
        nc.scalar.dma_start(
            out=mask_sb[:, :, :],
            in_=dram_ap(y1mask, 0, [[0, CMID], [66, 10], [1, 66]]),
        )
        comp_all = const.tile([128, 2, CMID], bf16, tag="comp")
        nc.scalar.dma_start(
            out=comp_all[:, :, :],
            in_=dram_ap(comp_wt, 0, [[CMID, 128], [128 * CMID, 2], [1, CMID]]),
        )
        enc_all = const.tile([CMID, 9, CENCP], bf16, tag="enc")
        nc.scalar.dma_start(
            out=enc_all[:, :, :],
            in_=dram_ap(enc_wt, 0, [[CENCP, CMID], [CMID * CENCP, 9], [1, CENCP]]),
        )

        # ---- batchnorm fold: inv = gamma/sqrt(var+eps), shift = beta-mean*inv
        def bn_fold(gamma, beta, mean, var, n, tagp):
            g = const.tile([n, 1], f32, tag=f"{tagp}g")
            bt = const.tile([n, 1], f32, tag=f"{tagp}b")
            m = const.tile([n, 1], f32, tag=f"{tagp}m")
            v = const.tile([n, 1], f32, tag=f"{tagp}v")
            for t, src in ((g, gamma), (bt, beta), (m, mean), (v, var)):
                nc.scalar.dma_start(out=t[:, :], in_=dram_ap(src, 0, [[1, n]]))
            eps = const.tile([n, 1], f32, tag=f"{tagp}e")
            nc.vector.memset(eps[:, :], 1e-5)
            std = const.tile([n, 1], f32, tag=f"{tagp}s")
            nc.scalar.activation(std[:, :], v[:, :], AF.Sqrt, bias=eps[:, :])
            rstd = const.tile([n, 1], f32, tag=f"{tagp}r")
            nc.vector.reciprocal(rstd[:, :], std[:, :])
            inv = const.tile([n, 1], f32, tag=f"{tagp}i")
            nc.vector.tensor_mul(inv[:, :], g[:, :], rstd[:, :])
            tmp = const.tile([n, 1], f32, tag=f"{tagp}t")
            nc.vector.tensor_mul(tmp[:, :], m[:, :], inv[:, :])
            shift = const.tile([n, 1], f32, tag=f"{tagp}h")
            nc.vector.tensor_sub(shift[:, :], bt[:, :], tmp[:, :])
            return inv, shift

        inv1, shift1 = bn_fold(c_g, c_b, c_m, c_v, CMID, "bn1")
        inv2, shift2 = bn_fold(e_g, e_b, e_m, e_v, CENCP, "bn2")

        # ---- p = clip(power_p, 1e-5), broadcast to [128,1] via DRAM bounce
        p_sb = const.tile([1, 1], f32, tag="p")
        nc.scalar.dma_start(out=p_sb[:, :], in_=dram_ap(p_in, 0, [[1, 1]]))
        nc.vector.tensor_scalar_max(p_sb[:, :], p_sb[:, :], 1e-5)
        p_wr = nc.scalar.dma_start(out=dram_ap(pscr, 0, [[1, 1]]), in_=p_sb[:, :])
        pb_sb = const.tile([CENCP, 1], f32, tag="pb")
        p_rd = nc.scalar.dma_start(
            out=pb_sb[:, :], in_=dram_ap(pscr, 0, [[0, CENCP], [1, 1]])
        )
        add_dep_helper(p_rd.ins, p_wr.ins, sync=True, reason="pscr RAW")

        # ---- Y1 tiles (zeroed once; borders stay zero) ----
        y1 = []
        for b in range(B):
            t = const.tile([CMID, 10, 66], bf16, tag=f"y1_{b}")
            nc.vector.memset(t[:, :, :], 0.0)
            y1.append(t)

        epad_wr = {}  # (b,ki) -> dma inst
        btcss = []

        # ===== PHASE 1 (per batch): conv -> softmax -> scatter =====
        for b in range(B):
            # conv1x1 + bn1 + relu
            for half in range(2):
                pc = ps_c1.tile([CMID, 320], f32, tag="c1")
                for ct in range(2):
                    nc.tensor.matmul(
                        pc[:, :],
                        comp_all[:, ct, :],
                        xbf[b][:, ct, 1 + 5 * half : 6 + 5 * half, 2 : 2 + W],
                        start=(ct == 0),
                        stop=(ct == 1),
                    )
                nc.scalar.activation(
                    y1[b][:, 5 * half : 5 * half + 5, 1 : 1 + W],
                    pc[:, :],
                    AF.Relu,
                    bias=shift1[:, :],
                    scale=inv1[:, :],
                )
            # zero out-of-image rows / padding cols
            nc.vector.tensor_mul(y1[b][:, :, :], y1[b][:, :, :], mask_sb[:, :, :])

            # conv3x3 + bn2
            pc3 = ps_c3.tile([CENCP, HS * W], f32, tag="c3")
            jj = 0
            for dy in (-1, 0, 1):
                for dx in (-1, 0, 1):
                    nc.tensor.matmul(
                        pc3[:, :],
                        enc_all[:, jj, :],
                        y1[b][:, 1 + dy : 9 + dy, 1 + dx : 1 + dx + W],
                        start=(jj == 0),
                        stop=(jj == 8),
                    )
                    jj += 1
            w_sb = sm.tile([CENCP, HS * W], f32, tag="w")
            nc.scalar.activation(
                w_sb[:, :], pc3[:, :], AF.Identity, bias=shift2[:, :], scale=inv2[:, :]
            )

            # power + softmax numerator (bf16 tail)
            nc.vector.tensor_scalar_max(w_sb[:, :], w_sb[:, :], 1e-5)
            nc.scalar.activation(w_sb[:, :], w_sb[:, :], AF.Ln)
            nc.scalar.activation(w_sb[:, :], w_sb[:, :], AF.Exp, scale=pb_sb[:, :])
            e_sb = sm.tile([CENCP, HS, W], bf16, tag="e")
            nc.scalar.activation(e_sb[:, :, :], w_sb[:, :], AF.Exp)

            # tap-sums (per subpixel s) -> 1/S via ACT LUT -> PE broadcast to
            # all 128 partitions (rb_ps[32s+k, :] = 1/S[s, :])
            ps = ps_s.tile([4, HS * W], f32, tag="s")
            nc.tensor.matmul(
                ps[:, :], sel_sb[:, :], e_sb[:, :, :], start=True, stop=True
            )
            l4 = sm.tile([4, HS * W], f32, tag="l4")
            nc.scalar.activation(l4[:, :], ps[:, :], AF.Ln)
            r4 = sm.tile([4, HS * W], bf16, tag="r4")
            nc.scalar.activation(r4[:, :], l4[:, :], AF.Exp, scale=-1.0)
            rb_ps = ps_rb.tile([CENCP, HS, W], f32, tag="rb")
            nc.tensor.matmul(
                rb_ps[:, :, :], bc4_sb[:, :], r4[:, :], start=True, stop=True
            )

            # normalized weights folded to [25 taps, (x, s, h)] layout
            en25 = sm.tile([25, W, 4, HS], bf16, tag="en25")
            for s in range(4):
                nc.vector.tensor_mul(
                    en25[:, :, s, :],
                    e_sb[32 * s : 32 * s + 25, :, :].rearrange("p h x -> p x h"),
                    rb_ps[32 * s : 32 * s + 25, :, :].rearrange("p h x -> p x h"),
                )

            # scatter to banded DRAM scratch (64B runs: (s,h) innermost)
            for ki in range(K_UP):
                wr = nc.sync.dma_start(
                    out=dram_ap(
                        epad,
                        (b * K_UP + ki) * EPK,
                        [[2048, K_UP], [2080, W], [1, 32]],
                    ),
                    in_=en25[5 * ki : 5 * (ki + 1), :, :, :],
                )
                epad_wr[(b, ki)] = wr

            # read the banded blocks back (contiguous 4KB runs) and repack
            # (x,s,h) -> (h,x,s) on-chip so the matmul rhs is contiguous
            btcs = []
            for ki in range(K_UP):
                bt = bp.tile([WP, W, 4, HS], bf16, tag="bt")
                rd = nc.sync.dma_start(
                    out=bt[:, :, :, :],
                    in_=dram_ap(
                        epad,
                        (b * K_UP + ki) * EPK,
                        [[2048, WP], [1, 2048]],
                    ),
                )
                add_dep_helper(
                    rd.ins, epad_wr[(b, ki)].ins, sync=True, reason="epad RAW"
                )
                btc = bc.tile([WP, HS, W, 4], bf16, tag=f"btc{b}{ki}")
                nc.any.tensor_copy(
                    btc[:, :, :, :],
                    bt[:, :, :, :].rearrange("p x s h -> p h x s"),
                )
                btcs.append(btc)
            btcss.append(btcs)

        # ===== PHASE 2 (per batch): banded matmuls -> output =====
        for b in range(B):
            btcs = btcss[b]
            for ct in range(2):
                o_all = op.tile([128, HS, 2, W, 2], f32, tag="oall")
                for h in range(HS):
                    pe = ps_e.tile([128, 256], f32, tag="pe")
                    for ki in range(K_UP):
                        nc.tensor.matmul(
                            pe[:, :],
                            xts[b][:, h + ki, ct * 128 : (ct + 1) * 128],
                            btcs[ki][:, h, :, :],
                            start=(ki == 0),
                            stop=(ki == K_UP - 1),
                        )
                    # psum cols are (x,ry,rx); store as (ry,x,rx)
                    nc.any.tensor_copy(
                        o_all[:, h, :, :, :],
                        pe[:, :].rearrange("p (x ry rx) -> p ry x rx", ry=2, rx=2),
                    )
                    if h == 3 or h == 5 or h == 7:
                        lo = 0 if h == 3 else (4 if h == 5 else 6)
                        n = 4 if h == 3 else 2
                        nc.scalar.dma_start(
                            out=dram_ap(
                                out,
                                (b * C + ct * 128) * 2 * HS * 2 * W + lo * 256,
                                [[2 * HS * 2 * W, 128], [1, n * 256]],
                            ),
                            in_=o_all[:, lo : lo + n, :, :, :],
                        )

        ctx.close()

    return nc


def _get_nc():
    if "nc" not in _STATE:
        _STATE["nc"] = _build_nc()
    return _STATE["nc"]


def _make_in_maps(inputs):
    X = np.asarray(inputs["X"], dtype=np.float32)
    Xp = np.pad(X, ((0, 0), (0, 0), (2, 2), (2, 2)))

    # enc output channels reordered+padded: new row c' = 32*s + k holds old
    # channel 4*k + s for k < 25; rows with k >= 25 are zero padding.
    def _pad_enc(a, fill=0.0):
        shp = (CENCP,) + a.shape[1:]
        o = np.full(shp, fill, np.float32)
        for s in range(4):
            for k in range(25):
                o[32 * s + k] = a[4 * k + s]
        return o

    sel = np.zeros((CENCP, 4), np.float32)
    for s in range(4):
        sel[32 * s : 32 * s + 25, s] = 1.0
    bc4 = np.zeros((4, 128), np.float32)
    for s in range(4):
        bc4[s, 32 * s : 32 * (s + 1)] = 1.0

    # lhsT weight layouts, bf16
    cw = np.asarray(inputs["comp_w"], np.float32)[:, :, 0, 0]   # [64, 256]
    comp_wt = np.ascontiguousarray(cw.T.reshape(2, 128, CMID)).astype(
        ml_dtypes.bfloat16
    )
    ew = _pad_enc(np.asarray(inputs["enc_w"], np.float32))      # [128, 64, 3, 3]
    enc_wt = np.ascontiguousarray(
        ew.reshape(CENCP, CMID, 9).transpose(2, 1, 0)
    ).astype(ml_dtypes.bfloat16)                                # [9, 64, 128]

    common = {
        "comp_wt": comp_wt,
        "comp_gamma": np.asarray(inputs["comp_gamma"], np.float32),
        "comp_beta": np.asarray(inputs["comp_beta"], np.float32),
        "comp_mean": np.asarray(inputs["comp_mean"], np.float32),
        "comp_var": np.asarray(inputs["comp_var"], np.float32),
        "enc_wt": enc_wt,
        "enc_gamma": _pad_enc(np.asarray(inputs["enc_gamma"], np.float32), fill=1.0),
        "enc_beta": _pad_enc(np.asarray(inputs["enc_beta"], np.float32)),
        "enc_mean": _pad_enc(np.asarray(inputs["enc_mean"], np.float32)),
        "enc_var": _pad_enc(np.asarray(inputs["enc_var"], np.float32), fill=1.0),
        "power_p": np.asarray(inputs["power_p"], np.float32),
        "sel": sel.astype(ml_dtypes.bfloat16),
        "bc4": bc4.astype(ml_dtypes.bfloat16),
    }
    in_maps = []
    for core in range(N_CORES):
        r0 = HS * core
        win = Xp[:, :, r0 : r0 + XROWS, :]
        xhv = np.ascontiguousarray(win).astype(ml_dtypes.bfloat16)
        xtv = np.ascontiguousarray(win.transpose(0, 2, 3, 1)).astype(
            ml_dtypes.bfloat16
        )
        mask = np.zeros((10, 66), np.float32)
        for rr in range(10):
            grow = r0 - 1 + rr
            if 0 <= grow < H:
                mask[rr, 1 : 1 + W] = 1.0
        m = dict(common)
        m["xh"] = xhv.reshape(B, 2, 128, XROWS * WP)
        m["xt"] = xtv
        m["y1mask"] = mask.reshape(660).astype(ml_dtypes.bfloat16)
        in_maps.append(m)
    return in_maps


def _run(inputs, trace=False):
    from concourse.bass_utils import run_bass_kernel_spmd

    if trace:
        import sys, os
        sys.path.insert(0, os.path.dirname(os.path.abspath(__file__)))
        import hookshim  # noqa: F401

    nc = _get_nc()
    in_maps = _make_in_maps(inputs)
    res = run_bass_kernel_spmd(
        nc, in_maps, core_ids=list(range(N_CORES)), trace=trace
    )
    out = np.concatenate([res.results[c]["out"] for c in range(N_CORES)], axis=2)
    return out, res


def kernel(**inputs):
    out, _ = _run(inputs, trace=False)
    return out
